# revision 1
# baseline (speedup 1.0000x reference)
"""Trainium2 Bass kernel for a hyperbolic (Mobius/expmap residual) transformer block.

Sharding: 8 cores = 2 (batch) x 4 (head groups of 4 heads / 256 channels).
Cores 0-3 handle batch 0, cores 4-7 batch 1; replica groups [[0..3],[4..7]].
Per core: LN1 -> PE transpose -> QKV (bf16 matmuls) -> causal attention in
score-transposed layout (softmax denominator via an appended ones-row on V,
no max subtraction: |scores| <= ~4) -> attn proj partial -> ReduceScatter
-> per-head hyperbolic expmap on own 256 cols -> AllGather -> LN2 -> FC+GELU
-> MLP proj partial -> ReduceScatter -> expmap -> per-core (2048, 256) slice.

All collectives are chunked 4x along tokens and pipelined against compute.
Host folds ln gains + 1/sqrt(hs) into weights and pre-transposes all weights.
"""

import numpy as np
import ml_dtypes

import concourse.bass as bass
import concourse.tile as tile
import concourse.mybir as mybir
from concourse.bass_utils import run_bass_kernel_spmd
from concourse.masks import make_identity
from concourse import bacc

F32 = mybir.dt.float32
BF16 = mybir.dt.bfloat16
AF = mybir.ActivationFunctionType
ALU = mybir.AluOpType

B, T, C = 2, 2048, 1024
H_TOT, H_LOC = 16, 4          # heads total / per core
HS = C // H_TOT               # 64
GC = H_LOC * HS               # 256 own channels per core
NT = T // 128                 # 16 token blocks
NC8 = C // 128                # 8 channel tiles
NTC = T // 512                # 4 token chunks of 512 (= collective chunks)
EPS = 1e-9
LN_EPS = 1e-5

_CACHE = {}


def build(debug=False, comm=True):
    nc = bacc.Bacc("TRN2", target_bir_lowering=False, debug=False, num_devices=8)

    x_d = nc.dram_tensor("x", [T, C], F32, kind="ExternalInput")
    xown_d = nc.dram_tensor("xown", [T, GC], F32, kind="ExternalInput")
    wqkvT_d = nc.dram_tensor("wqkvT", [C, 3 * GC], BF16, kind="ExternalInput")
    wpT_d = nc.dram_tensor("wpT", [GC, C], BF16, kind="ExternalInput")
    wfcT_d = nc.dram_tensor("wfcT", [C, C], BF16, kind="ExternalInput")
    wmpT_d = nc.dram_tensor("wmpT", [C, C], BF16, kind="ExternalInput")
    cst_d = nc.dram_tensor("cst", [128, 2, 4, H_LOC], F32, kind="ExternalInput")
    mask_d = nc.dram_tensor("mask", [128, 128], BF16, kind="ExternalInput")
    out_d = nc.dram_tensor("out", [T, GC], F32, kind="ExternalOutput")
    dbg = {}
    if debug:
        for nm, shp in [("d_qH", [64, H_LOC, T]), ("d_kH", [64, H_LOC, T]),
                        ("d_vaug", [128, NT, 4 * 65]), ("d_yT", [128, 2, T]),
                        ("d_aown", [T, GC]), ("d_x2own", [T, GC]),
                        ("d_hown", [T, GC]), ("d_ln1T", [128, NC8, T]),
                        ("d_mv", [T, 2])]:
            dbg[nm] = nc.dram_tensor(nm, shp, F32, kind="ExternalOutput")

    with tile.TileContext(nc) as tc:
        _body(nc, tc, x_d, xown_d, wqkvT_d, wpT_d, wfcT_d, wmpT_d, cst_d,
              mask_d, out_d, dbg, comm)
    nc.compile()
    return nc


def _body(nc, tc, x_d, xown_d, wqkvT_d, wpT_d, wfcT_d, wmpT_d, cst_d, mask_d,
          out_d, dbg, comm=True):
    from contextlib import ExitStack
    ctx = ExitStack()
    pool = lambda name, bufs, space="SBUF": ctx.enter_context(
        tc.tile_pool(name=name, bufs=bufs, space=space))

    consts = pool("consts", 1)
    wbig = pool("wbig", 2)          # wqk/wv -> wfc halves -> wmp halves
    wp_p = pool("wp", 1)
    bigT = pool("bigT", 2)          # ln1T -> ln2T -> hT
    attn = pool("attn", 1)          # qH, kH, yT128, V_aug
    x2o_p = pool("x2own", 1)
    xio = pool("xio", 2)            # [128,1024] f32 loads
    lnb_p = pool("lnb", 2)          # bf16 ln tiles
    exp_p = pool("expp", 4)
    acp = pool("acp", 3)            # f32 [128,512] copies to bounce
    sm = pool("sm", 2)              # small transient tiles
    stg = pool("stg", 3)            # [128,512] bf16 partition-shift staging
    rb_p = pool("rb", 2)
    den_p = pool("den", 2)
    chain = pool("chain", 1)        # expmap chain [128, 4, 4] per chunk
    dram = pool("dram", 1, "DRAM")
    psum = pool("psum", 1, "PSUM")

    # DMA issue spread: bulk transfers via Pool (SWDGE, ~free sequencer) and
    # SP; scalar engine left for compute.
    def dma(dst, src):
        return nc.sync.dma_start(dst, src)

    def cp(dst, src):
        return nc.vector.tensor_copy(dst, src)

    # ---- constants ----
    identb = consts.tile([128, 128], BF16)
    make_identity(nc, identb[:])
    maskb = consts.tile([128, 128], BF16)
    nc.sync.dma_start(maskb[:], mask_d.ap())
    cst = consts.tile([128, 2, 4, H_LOC], F32)
    nc.sync.dma_start(cst[:], cst_d.ap())
    ones64 = consts.tile([1, 64], BF16)
    nc.vector.memset(ones64[:], 1.0)
    eps5 = consts.tile([128, 1], F32)
    nc.vector.memset(eps5[:], LN_EPS)
    eps9 = consts.tile([128, 1], F32)
    nc.vector.memset(eps9[:], EPS)

    # ---- DRAM bounce buffers (per token-chunk of 512 for pipelining) ----
    rs1_in = [dram.tile([4, 512, GC], F32, name=f"rs1i{c}") for c in range(NTC)]
    rs1_out = [dram.tile([512, GC], F32, name=f"rs1o{c}") for c in range(NTC)]
    ag_in = [dram.tile([512, GC], F32, name=f"agi{c}") for c in range(NTC)]
    ag_out = [dram.tile([4, 512, GC], F32, name=f"ago{c}") for c in range(NTC)]
    rs2_in = [dram.tile([4, 512, GC], F32, name=f"rs2i{c}") for c in range(NTC)]
    rs2_out = [dram.tile([512, GC], F32, name=f"rs2o{c}") for c in range(NTC)]
    GROUPS = [[0, 1, 2, 3], [4, 5, 6, 7]]

    def do_rs(src_t, dst_t):
        if comm:
            nc.gpsimd.collective_compute(
                "ReduceScatter", ALU.add, replica_groups=GROUPS,
                ins=[src_t.opt()], outs=[dst_t.opt()])
        else:
            nc.sync.dma_start(dst_t[:], src_t[0, :, :])

    def do_ag(src_t, dst_t):
        if comm:
            nc.gpsimd.collective_compute(
                "AllGather", ALU.bypass, replica_groups=GROUPS,
                ins=[src_t.opt()], outs=[dst_t.opt()])
        else:
            for gg in range(4):
                nc.sync.dma_start(dst_t[gg, :, :], src_t[:])

    # ---- persistent SBUF ----
    wqk = wbig.tile([128, NC8, 512], BF16, tag="w")
    dma(wqk[:], wqkvT_d.ap()[:, 0:512].rearrange("(a p) o -> p a o", p=128))
    wv = wbig.tile([128, NC8, GC], BF16, tag="w")
    dma(wv[:], wqkvT_d.ap()[:, 512:768].rearrange("(a p) o -> p a o", p=128))
    wpT = wp_p.tile([128, 2, C], BF16)
    dma(wpT[:], wpT_d.ap().rearrange("(a p) o -> p a o", p=128))

    ln1T = bigT.tile([128, NC8, T], BF16, tag="bigT")
    qH = attn.tile([64, H_LOC, T], BF16)
    kH = attn.tile([64, H_LOC, T], BF16)
    yT128 = attn.tile([128, 2, T], BF16)
    V_aug = attn.tile([128, NT, H_LOC * 65], BF16)
    _va = V_aug[:]
    nc.vector.memset(bass.AP(tensor=_va.tensor, offset=_va.offset + 64,
                             ap=[_va.ap[0], [H_LOC * 65, NT], [65, H_LOC]]),
                     1.0)
    x2own = x2o_p.tile([128, NT, GC], F32)

    def layernorm_transpose(src_load, dstT, chunk=0, mv_dbg=None,
                            fast_start=False):
        # pass 1: stats for the 4 t-blocks, batched rsqrt (one ACT op/chunk);
        # fast_start computes rsqrt per block so the first transpose can
        # begin before all four blocks' stats are in (pipeline ramp).
        xts = []
        mv_b = sm.tile([128, 4, 2], F32, tag="bnmv", bufs=2)
        r_b = sm.tile([128, 4], F32, tag="rt", bufs=2)
        for tbl in range(4):
            tb = 4 * chunk + tbl
            x_t = src_load(tb)
            xts.append(x_t)
            st = sm.tile([128, 2, 6], F32, tag="bnst", bufs=3)
            nc.vector.bn_stats(st[:, 0, :], x_t[:, 0:512])
            nc.vector.bn_stats(st[:, 1, :], x_t[:, 512:1024])
            nc.vector.bn_aggr(mv_b[:, tbl, :], st[:])
            if mv_dbg is not None:
                nc.sync.dma_start(mv_dbg.ap()[tb * 128:(tb + 1) * 128, :],
                                  mv_b[:, tbl, :])
            if fast_start:
                sd1 = sm.tile([128, 1], F32, tag="sd", bufs=2, name="sd1")
                nc.scalar.activation(sd1[:], mv_b[:, tbl, 1:2], AF.Sqrt,
                                     bias=eps5[:])
                nc.vector.reciprocal(r_b[:, tbl:tbl + 1], sd1[:])
        if not fast_start:
            sd_b = sm.tile([128, 4], F32, tag="sd", bufs=2)
            nc.scalar.activation(sd_b[:], mv_b[:, :, 1], AF.Sqrt,
                                 bias=eps5[:])
            nc.vector.reciprocal(r_b[:], sd_b[:])
        for tbl in range(4):
            tb = 4 * chunk + tbl
            x_t = xts[tbl]
            lnb = lnb_p.tile([128, C], BF16, tag="lnb", bufs=2)
            nc.vector.tensor_scalar(lnb[:], x_t[:], mv_b[:, tbl, 0:1],
                                    r_b[:, tbl:tbl + 1],
                                    ALU.subtract, ALU.mult)
            for ctq in range(2):
                tp = psum.tile([128, 4, 128], BF16, tag="tr", bufs=1)
                for k in range(4):
                    ct = 4 * ctq + k
                    nc.tensor.transpose(tp[:, k, :],
                                        lnb[:, ct * 128:(ct + 1) * 128],
                                        identb[:])
                cp(dstT[:, 4 * ctq:4 * ctq + 4, tb * 128:(tb + 1) * 128],
                   tp[:])

    # ================= P1: LN1 + transpose =================
    def load_x(tb):
        x_t = xio.tile([128, C], F32, tag="xio", bufs=4)
        dma(x_t[:], x_d.ap()[tb * 128:(tb + 1) * 128, :])
        return x_t

    def st_ln1(c):
        layernorm_transpose(load_x, ln1T, chunk=c, mv_dbg=dbg.get("d_mv"),
                            fast_start=(c == 0))

    # ================= P2: QKV =================
    def st_qkv(c):
        tcn = c
        for ot in range(4):              # q01 q23 k01 k23
            dst, hpair = (qH, ot) if ot < 2 else (kH, ot - 2)
            ps = psum.tile([128, 512], F32, tag="stream", bufs=2)
            for ct in range(NC8):
                nc.tensor.matmul(
                    ps[:], wqk[:, ct, ot * 128:(ot + 1) * 128],
                    ln1T[:, ct, tcn * 512:(tcn + 1) * 512],
                    start=(ct == 0), stop=(ct == NC8 - 1))
            sl = slice(tcn * 512, (tcn + 1) * 512)
            s_t = stg.tile([128, 512], BF16, tag="stg", bufs=2)
            nc.scalar.copy(s_t[:], ps[:])
            dma(dst[:, 2 * hpair, sl], s_t[0:64, :])
            dma(dst[:, 2 * hpair + 1, sl], s_t[64:128, :])
        # V for this chunk's 4 t-blocks
        for tb in range(4 * c, 4 * c + 4):
            ps = psum.tile([128, 256], F32, tag="stream", bufs=2)
            for ct in range(NC8):
                nc.tensor.matmul(ps[:], ln1T[:, ct, tb * 128:(tb + 1) * 128],
                                 wv[:, ct, :],
                                 start=(ct == 0), stop=(ct == NC8 - 1))
            vdst = V_aug[:, tb, :]
            vap = bass.AP(tensor=vdst.tensor, offset=vdst.offset,
                          ap=[vdst.ap[0], [65, H_LOC], [1, 64]])
            nc.vector.tensor_copy(
                vap, ps[:].rearrange("p (h d) -> p h d", h=H_LOC))

    # ================= P5: expmap (chunked over 4 t-blocks) ================
    def expmap_chunk(ch, v_load, x_load, phase, out_write, dve_only=False):
        """out = expmap(x, v) per head for t-blocks 4ch..4ch+3."""
        cc = cst[:, phase, 0, :]
        twoc = cst[:, phase, 1, :]
        ccsq = cst[:, phase, 2, :]
        isc = cst[:, phase, 3, :]
        LONG = {"xns", "vns", "ipr", "t1", "s_", "yn", "al1", "al12",
                "alpha", "gamma", "alr", "gar"}

        def q(nm):
            tag = nm if nm in LONG else "chtmp"
            return chain.tile([128, 4, H_LOC], F32, tag=tag, name=nm,
                              bufs=2 if nm in LONG else 8)
        XNS, VNS, IPR = q("xns"), q("vns"), q("ipr")
        for tbl in range(4):
            x_t = x_load(tbl)
            v_t = v_load(tbl)
            meng = nc.vector if dve_only else nc.gpsimd
            sq = sm.tile([128, GC], F32, tag="sq", bufs=3)
            meng.tensor_tensor(out=sq[:], in0=x_t[:], in1=x_t[:],
                               op=ALU.mult)
            nc.vector.tensor_reduce(
                XNS[:, tbl, :], sq[:].rearrange("p (h d) -> p h d", h=H_LOC),
                axis=mybir.AxisListType.X, op=ALU.add)
            sq2 = sm.tile([128, GC], F32, tag="sq", bufs=3, name="sq2")
            meng.tensor_tensor(out=sq2[:], in0=v_t[:], in1=v_t[:],
                               op=ALU.mult)
            nc.vector.tensor_reduce(
                VNS[:, tbl, :], sq2[:].rearrange("p (h d) -> p h d", h=H_LOC),
                axis=mybir.AxisListType.X, op=ALU.add)
            pq = sm.tile([128, GC], F32, tag="sq", bufs=3, name="pq")
            meng.tensor_tensor(out=pq[:], in0=x_t[:], in1=v_t[:],
                               op=ALU.mult)
            nc.vector.tensor_reduce(
                IPR[:, tbl, :], pq[:].rearrange("p (h d) -> p h d", h=H_LOC),
                axis=mybir.AxisListType.X, op=ALU.add)

        def bcst(ap_):  # broadcast [128,4] over the 4 t-blocks
            return bass.AP(tensor=ap_.tensor, offset=ap_.offset,
                           ap=[ap_.ap[0], [0, 4], ap_.ap[-1]])
        tt = lambda o, a, b_: nc.vector.tensor_tensor(out=o, in0=a, in1=b_,
                                                      op=ALU.mult)
        ta = lambda o, a, b_: nc.vector.tensor_tensor(out=o, in0=a, in1=b_,
                                                      op=ALU.add)
        flat = lambda a: a[:].rearrange("p a b -> p (a b)")
        t1 = q("t1"); tt(t1[:], XNS[:], bcst(cc))
        u1 = q("u1"); nc.vector.tensor_scalar_add(u1[:], t1[:], 1.0 + EPS)
        r1 = q("r1"); nc.vector.reciprocal(r1[:], u1[:])
        u2 = q("u2"); tt(u2[:], VNS[:], bcst(cc))
        u3 = q("u3"); tt(u3[:], u2[:], r1[:])
        s1 = q("s1")
        nc.scalar.activation(flat(s1), flat(u3), AF.Sqrt, bias=eps9[:])
        vn = q("vn")
        nc.scalar.activation(flat(vn), flat(VNS), AF.Sqrt, bias=eps9[:])
        th = q("th")
        nc.scalar.activation(flat(th), flat(s1), AF.Tanh)
        coeff = q("coeff"); tt(coeff[:], th[:], bcst(isc))
        u4 = q("u4"); nc.vector.tensor_scalar_add(u4[:], vn[:], EPS)
        r2 = q("r2"); nc.vector.reciprocal(r2[:], u4[:])
        s_ = q("s_"); tt(s_[:], coeff[:], r2[:])
        ip = q("ip"); tt(ip[:], s_[:], IPR[:])
        s2 = q("s2"); tt(s2[:], s_[:], s_[:])
        yn = q("yn"); tt(yn[:], s2[:], VNS[:])
        al1 = q("al1"); tt(al1[:], ip[:], bcst(twoc))
        al2 = q("al2"); tt(al2[:], yn[:], bcst(cc))
        al12 = q("al12"); ta(al12[:], al1[:], al2[:])
        alpha = q("alpha"); nc.vector.tensor_scalar_add(alpha[:], al12[:], 1.0)
        beta = q("beta")
        nc.vector.tensor_scalar(beta[:], t1[:], -1.0, 1.0, ALU.mult, ALU.add)
        gamma = q("gamma"); tt(gamma[:], beta[:], s_[:])
        d1 = q("d1"); tt(d1[:], XNS[:], bcst(ccsq))
        d2 = q("d2"); tt(d2[:], d1[:], yn[:])
        dd = q("dd"); ta(dd[:], al1[:], d2[:])
        den_e = q("den_e")
        nc.vector.tensor_scalar_add(den_e[:], dd[:], 1.0 + EPS)
        rden = q("rden"); nc.vector.reciprocal(rden[:], den_e[:])
        alr = q("alr"); tt(alr[:], alpha[:], rden[:])
        gar = q("gar"); tt(gar[:], gamma[:], rden[:])

        def bch(ap_, tbl):  # [128,4] slice -> [128, 4, HS] free-bcast
            sl_ = ap_[:, tbl, :]
            return bass.AP(tensor=sl_.tensor, offset=sl_.offset,
                           ap=[sl_.ap[0], sl_.ap[-1], [0, HS]])
        for tbl in range(4):
            x_t = x_load(tbl)
            v_t = v_load(tbl)
            o1 = sm.tile([128, GC], F32, tag="o1", bufs=2)
            nc.vector.tensor_tensor(
                out=o1[:].rearrange("p (h d) -> p h d", h=H_LOC),
                in0=x_t[:].rearrange("p (h d) -> p h d", h=H_LOC),
                in1=bch(alr, tbl), op=ALU.mult)
            o2 = sm.tile([128, GC], F32, tag="o2", bufs=2)
            nc.vector.tensor_tensor(
                out=o2[:].rearrange("p (h d) -> p h d", h=H_LOC),
                in0=v_t[:].rearrange("p (h d) -> p h d", h=H_LOC),
                in1=bch(gar, tbl), op=ALU.mult)
            out_write(tbl, o1, o2)

    # ================= P3+P4: attention + proj + RS1 =================
    def st_attn(j):
        for h in range(H_LOC):
            pv = psum.tile([65, 512], F32, tag="pv", bufs=2)
            nblk = 4 * j + 4
            for i in range(nblk):               # tk block of 128
                r = i - 4 * j
                lo = max(0, r * 128)
                sc = psum.tile([128, 512], F32, tag="sc", bufs=2)
                nc.tensor.matmul(
                    sc[:, lo:512],
                    kH[:, h, i * 128:(i + 1) * 128],
                    qH[:, h, j * 512 + lo:(j + 1) * 512],
                    start=True, stop=True)
                ex = exp_p.tile([128, 512], BF16, tag="exp", bufs=4)
                nc.scalar.activation(ex[:, lo:512], sc[:, lo:512], AF.Exp)
                if r >= 0:
                    nc.gpsimd.tensor_tensor(
                        out=ex[:, lo:lo + 128], in0=ex[:, lo:lo + 128],
                        in1=maskb[:], op=ALU.mult)
                nc.tensor.matmul(pv[:, lo:512],
                                 V_aug[:, i, 65 * h:65 * h + 65],
                                 ex[:, lo:512],
                                 start=(i == 0), stop=(i == nblk - 1))
            # normalize: den = pv[64]; yT = pv[0:64] / den
            d_sb = den_p.tile([65, 512], BF16, tag="dsb", bufs=2)
            nc.vector.tensor_copy(d_sb[64:65, :], pv[64:65, :])
            d0 = den_p.tile([1, 512], BF16, tag="d0", bufs=2)
            nc.sync.dma_start(d0[:], d_sb[64:65, :])
            rr = den_p.tile([1, 512], BF16, tag="rr", bufs=2)
            with nc.allow_low_precision(reason="softmax denom recip bf16"):
                nc.vector.reciprocal(rr[:], d0[:])
            bc = psum.tile([64, 512], F32, tag="bc", bufs=1)
            nc.tensor.matmul(bc[:], ones64[:], rr[:], start=True, stop=True)
            rb = rb_p.tile([64, 512], F32, tag="rb", bufs=2)
            nc.vector.tensor_copy(rb[:], bc[:])
            sl = slice(j * 512, (j + 1) * 512)
            if h % 2 == 0:
                nc.vector.tensor_tensor(out=yT128[0:64, h // 2, sl],
                                        in0=pv[0:64, :], in1=rb[:],
                                        op=ALU.mult)
            else:
                s_t = stg.tile([128, 512], BF16, tag="stg", bufs=2)
                nc.vector.tensor_tensor(out=s_t[0:64, :], in0=pv[0:64, :],
                                        in1=rb[:], op=ALU.mult)
                dma(yT128[64:128, h // 2, sl], s_t[0:64, :])

    def proj_chunk(lhsT_tile, nk, rhs_of_oc, bounce, j, use_act_cp=False):
        """proj for token chunk j (4 t-blocks); bounce is (4, 512, GC)."""
        for oc in range(2):
            rhs_tile, osl = rhs_of_oc(oc)
            for tbl in range(4):
                tb = 4 * j + tbl
                ps = psum.tile([128, 512], F32, tag="stream", bufs=2)
                for kc in range(nk):
                    nc.tensor.matmul(
                        ps[:], lhsT_tile[:, kc, tb * 128:(tb + 1) * 128],
                        rhs_tile[:, kc, osl],
                        start=(kc == 0), stop=(kc == nk - 1))
                a_t = acp.tile([128, 512], F32, tag="acp", bufs=3)
                if use_act_cp:
                    nc.scalar.copy(a_t[:], ps[:])
                else:
                    nc.vector.tensor_copy(a_t[:], ps[:])
                g0 = oc * 2
                tgt = bass.AP(
                    tensor=bounce.tensor,
                    offset=bounce[:].offset + g0 * 512 * GC + tbl * 128 * GC,
                    ap=[[GC, 128], [512 * GC, 2], [1, GC]])
                dma(tgt, a_t[:].rearrange("p (g o) -> p g o", g=2))

    wp_rhs = lambda oc: (wpT, slice(oc * 512, (oc + 1) * 512))

    def st_proj1(j):
        proj_chunk(yT128, 2, wp_rhs, rs1_in[j], j, use_act_cp=True)
        do_rs(rs1_in[j], rs1_out[j])

    def st_exp1(ch):
        def load_a1(tbl, _ch=ch):
            a_t = sm.tile([128, GC], F32, tag="a1", bufs=2)
            dma(a_t[:], rs1_out[_ch][tbl * 128:(tbl + 1) * 128, :])
            return a_t

        def load_xown(tbl, _ch=ch):
            x_t = sm.tile([128, GC], F32, tag="xo", bufs=2)
            dma(x_t[:], xown_d.ap()[(4 * _ch + tbl) * 128:
                                    (4 * _ch + tbl + 1) * 128, :])
            return x_t

        def write_x2(tbl, o1, o2, _ch=ch):
            tb = 4 * _ch + tbl
            nc.gpsimd.tensor_tensor(out=x2own[:, tb, :], in0=o1[:],
                                    in1=o2[:], op=ALU.add)
            dma(ag_in[_ch][tbl * 128:(tbl + 1) * 128, :], x2own[:, tb, :])

        expmap_chunk(ch, load_a1, load_xown, 0, write_x2)
        do_ag(ag_in[ch], ag_out[ch])





    # ================= P6: LN2 + transpose =================
    ln2T = bigT.tile([128, NC8, T], BF16, tag="bigT")

    def load_x2(tb):
        x_t = xio.tile([128, C], F32, tag="xio", bufs=4)
        ch, tbl = divmod(tb, 4)
        src = bass.AP(tensor=ag_out[ch].tensor,
                      offset=ag_out[ch][:].offset + tbl * 128 * GC,
                      ap=[[GC, 128], [512 * GC, 4], [1, GC]])
        dma(x_t[:].rearrange("p (g o) -> p g o", g=4), src)
        return x_t

    def st_ln2(c):
        layernorm_transpose(load_x2, ln2T, chunk=c)

    # ================= P7+P8: FC + GELU + MLP proj + RS2 (per chunk) =======
    hT = bigT.tile([128, NC8, T], BF16, tag="bigT")
    def wmp_rhs(oc):
        wmpT = wbig.tile([128, NC8, 512], BF16, tag="w", name="wmpT")
        dma(wmpT[:], wmpT_d.ap()[:, oc * 512:(oc + 1) * 512]
            .rearrange("(a p) o -> p a o", p=128))
        return wmpT, slice(0, 512)

    def st_fcmlp(c):
        tcn = c
        for half in range(2):
            wfcT = wbig.tile([128, NC8, 512], BF16, tag="w", name="wfcT")
            dma(wfcT[:], wfcT_d.ap()[:, half * 512:(half + 1) * 512]
                .rearrange("(a p) o -> p a o", p=128))
            for otl in range(4):
                ot = half * 4 + otl
                ps = psum.tile([128, 512], F32, tag="stream", bufs=2)
                for ct in range(NC8):
                    nc.tensor.matmul(
                        ps[:], wfcT[:, ct, otl * 128:(otl + 1) * 128],
                        ln2T[:, ct, tcn * 512:(tcn + 1) * 512],
                        start=(ct == 0), stop=(ct == NC8 - 1))
                nc.scalar.activation(hT[:, ot, tcn * 512:(tcn + 1) * 512],
                                     ps[:], AF.Gelu)
        proj_chunk(hT, NC8, wmp_rhs, rs2_in[c], c)
        do_rs(rs2_in[c], rs2_out[c])

    def _expmap2_chunk(ch):
        def load_h(tbl, _ch=ch):
            h_t = sm.tile([128, GC], F32, tag="a1", bufs=2, name="h_t")
            dma(h_t[:], rs2_out[_ch][tbl * 128:(tbl + 1) * 128, :])
            return h_t

        def x2_view(tbl, _ch=ch):
            return x2own[:, 4 * _ch + tbl, :]

        def write_out(tbl, o1, o2, _ch=ch):
            o3 = sm.tile([128, GC], F32, tag="o3", bufs=2)
            eng = nc.vector if _ch == NTC - 1 else nc.gpsimd
            eng.tensor_tensor(out=o3[:], in0=o1[:], in1=o2[:], op=ALU.add)
            dma(out_d.ap()[(4 * _ch + tbl) * 128:(4 * _ch + tbl + 1) * 128, :],
                o3[:])

        expmap_chunk(ch, load_h, x2_view, 1, write_out,
                     dve_only=(ch == NTC - 1))

    # ============ chunk-major software pipeline (diagonal wavefront) =======
    stages = [st_ln1, st_qkv, st_attn, st_proj1, st_exp1, st_ln2, st_fcmlp,
              _expmap2_chunk]
    for step in range(NTC + len(stages) - 1):
        for si, stf in enumerate(stages):
            c = step - si
            if 0 <= c < NTC:
                stf(c)

    if dbg:
        for nm, t in [("d_qH", qH), ("d_kH", kH)]:
            for hh in range(H_LOC):
                for tcn in range(NTC):
                    t32 = acp.tile([128, 512], F32, tag="acp", bufs=3)
                    nc.vector.tensor_copy(t32[0:64, :],
                                          t[:, hh, tcn * 512:(tcn + 1) * 512])
                    nc.sync.dma_start(
                        dbg[nm].ap()[:, hh, tcn * 512:(tcn + 1) * 512],
                        t32[0:64, :])
        for tb in range(NT):
            t32 = acp.tile([128, 512], F32, tag="acp", bufs=3)
            nc.vector.tensor_copy(t32[:, 0:260], V_aug[:, tb, :])
            nc.sync.dma_start(dbg["d_vaug"].ap()[:, tb, :], t32[:, 0:260])
        for kc in range(2):
            for tcn in range(NTC):
                t32 = acp.tile([128, 512], F32, tag="acp", bufs=3)
                nc.vector.tensor_copy(t32[:],
                                      yT128[:, kc, tcn * 512:(tcn + 1) * 512])
                nc.sync.dma_start(
                    dbg["d_yT"].ap()[:, kc, tcn * 512:(tcn + 1) * 512], t32[:])
    if dbg:
        for tb in range(NT):
            a_t = sm.tile([128, GC], F32, tag="a1", bufs=2)
            nc.sync.dma_start(a_t[:], rs1_out[tb // 4][(tb % 4) * 128:
                                                       (tb % 4 + 1) * 128, :])
            nc.sync.dma_start(dbg["d_aown"].ap()[tb * 128:(tb + 1) * 128, :],
                              a_t[:])
        for tb in range(NT):
            nc.sync.dma_start(dbg["d_x2own"].ap()[tb * 128:(tb + 1) * 128, :],
                              x2own[:, tb, :])

    if dbg:
        for tb in range(NT):
            h_t = sm.tile([128, GC], F32, tag="a1", bufs=2, name="h_t")
            nc.sync.dma_start(h_t[:], rs2_out[tb // 4][(tb % 4) * 128:
                                                       (tb % 4 + 1) * 128, :])
            nc.sync.dma_start(dbg["d_hown"].ap()[tb * 128:(tb + 1) * 128, :],
                              h_t[:])

    ctx.close()


# ===================== host side =====================

def _prep_inputs(inputs):
    x = np.asarray(inputs["x"], np.float32)
    g1 = np.asarray(inputs["ln1_g"], np.float32)
    wqkv = np.asarray(inputs["w_qkv"], np.float32)
    wap = np.asarray(inputs["w_attn_proj"], np.float32)
    cA = np.asarray(inputs["c_attn"], np.float32)
    g2 = np.asarray(inputs["ln2_g"], np.float32)
    wfc = np.asarray(inputs["w_fc"], np.float32)
    wmp = np.asarray(inputs["w_mlp_proj"], np.float32)
    cM = np.asarray(inputs["c_mlp"], np.float32)

    mask = np.triu(np.ones((128, 128), np.float32))  # keep tk <= tq
    in_maps = []
    for core in range(8):
        b, g = divmod(core, 4)
        qp = wqkv[g * GC:(g + 1) * GC, :] * g1[None, :] * (HS ** -0.5)
        kp = wqkv[C + g * GC:C + (g + 1) * GC, :] * g1[None, :]
        vp = wqkv[2 * C + g * GC:2 * C + (g + 1) * GC, :] * g1[None, :]
        wqkvT = np.ascontiguousarray(
            np.concatenate([qp, kp, vp], 0).T).astype(ml_dtypes.bfloat16)
        wpT = np.ascontiguousarray(
            wap[:, g * GC:(g + 1) * GC].T).astype(ml_dtypes.bfloat16)
        wfcT = np.ascontiguousarray(
            (wfc[g * C:(g + 1) * C, :] * g2[None, :]).T).astype(ml_dtypes.bfloat16)
        wmpT = np.ascontiguousarray(
            wmp[:, g * C:(g + 1) * C].T).astype(ml_dtypes.bfloat16)
        cst = np.zeros((2, 4, H_LOC), np.float32)
        for ph, cv in ((0, cA), (1, cM)):
            cc = np.clip(cv[g * H_LOC:(g + 1) * H_LOC], 1e-4, 1.0)
            cst[ph, 0] = cc
            cst[ph, 1] = 2 * cc
            cst[ph, 2] = cc * cc
            cst[ph, 3] = 1.0 / (np.sqrt(np.abs(cc) + EPS) + EPS)
        cst128 = np.broadcast_to(cst, (128, 2, 4, H_LOC)).copy()
        in_maps.append({
            "x": np.ascontiguousarray(x[b]),
            "xown": np.ascontiguousarray(x[b][:, g * GC:(g + 1) * GC]),
            "wqkvT": wqkvT, "wpT": wpT, "wfcT": wfcT, "wmpT": wmpT,
            "cst": cst128, "mask": mask.astype(ml_dtypes.bfloat16),
        })
    return in_maps


def kernel(debug=False, trace=False, **inputs):
    key = ("dbg" if debug else "run")
    if key not in _CACHE:
        _CACHE[key] = build(debug=debug)
    nc = _CACHE[key]
    in_maps = _prep_inputs(inputs)
    res = run_bass_kernel_spmd(nc, in_maps, core_ids=list(range(8)),
                               trace=trace)
    out = np.zeros((B, T, C), np.float32)
    for core in range(8):
        b, g = divmod(core, 4)
        out[b, :, g * GC:(g + 1) * GC] = res.results[core]["out"]
    if debug or trace:
        return out, res
    return out

